# Initial kernel scaffold
#
"""Trainium2 Bass kernel for Calc_Xi_And_LogLikelihood.

reference:
    s  = sigma + 1e-5
    xi = mu + s * eps
    z  = (xi - mu) / s          (== eps up to fp32 rounding noise ~1e-6 rel)
    LL[b,n] = sum_t [ -0.5*sum_d z^2 - sum_d log s - 0.5*D*log(2pi) ]

Sharding: data-parallel over the batch axis. Full shape (64,16,200,64);
each of the 8 cores gets 8 batch rows = a contiguous (128, 12800) f32 slab
(row = one (b,n) pair, free dim = TS*D). Row-wise reductions then map to
per-partition reductions. No cross-core communication.

Per core, per 1600-wide chunk of the free dim:
  DVE : prod = (sigma + 1e-5) * eps           (one scalar_tensor_tensor)
  DVE : xi   = prod + mu                      (tensor_add)      -> DMA out
  DVE : acc_sq[c]  = sum(-0.5 * eps * eps)    (tensor_tensor_reduce,
                                               out collapsed to a dummy column)
  ACT : acc_ln[c]  = sum(ln(sigma + 1e-5))    (activation Ln w/ accum_out)
Finally LL = (sum_c acc_sq - 200*0.5*D*log(2pi)) - sum_c acc_ln.

z = eps is numerically safe: checked against the f64 oracle, the shortcut is
~3.5e-8 max rel err while the fp32 reference itself carries ~1.6e-6.
"""

import math

import numpy as np

N_CORES = 8
BS, NCH, TS, D = 64, 16, 200, 64
ROWS_TOTAL = BS * NCH            # 1024 (b,n) rows overall
ROWS = ROWS_TOTAL // N_CORES     # 128 rows per core -> one SBUF partition each
F = TS * D                       # 12800 contiguous elements per row
FC = 1600                        # free-dim chunk
NCHUNK = F // FC                 # 8
LOG_NORM_TOTAL = TS * 0.5 * D * math.log(2.0 * math.pi)  # 11762.413225019807

_BUILT = None


def _build():
    import concourse.bass as bass
    import concourse.tile as tile
    from concourse import bacc, mybir

    f32 = mybir.dt.float32
    Alu = mybir.AluOpType

    nc = bacc.Bacc(
        "TRN2",
        target_bir_lowering=False,
        debug=False,
        enable_asserts=False,
        num_devices=N_CORES,
    )

    mu_d = nc.dram_tensor("mu", [ROWS, F], f32, kind="ExternalInput").ap()
    sg_d = nc.dram_tensor("sigma", [ROWS, F], f32, kind="ExternalInput").ap()
    ep_d = nc.dram_tensor("eps", [ROWS, F], f32, kind="ExternalInput").ap()
    xi_d = nc.dram_tensor("xi", [ROWS, F], f32, kind="ExternalOutput").ap()
    ll_d = nc.dram_tensor("ll", [ROWS, 1], f32, kind="ExternalOutput").ap()

    with tile.TileContext(nc) as tc:
        with (
            tc.tile_pool(name="io", bufs=3) as iop,
            tc.tile_pool(name="scr", bufs=2) as scr,
            tc.tile_pool(name="acc", bufs=1) as accp,
        ):
            bias_t = accp.tile([ROWS, 1], f32, tag="bias")
            nc.vector.memset(bias_t, 1e-5)
            sq_part = accp.tile([ROWS, NCHUNK], f32, tag="sqp")
            ln_part = accp.tile([ROWS, NCHUNK], f32, tag="lnp")
            dummy = accp.tile([ROWS, 1], f32, tag="dummy")

            for c in range(NCHUNK):
                sl = bass.ts(c, FC)
                mu_t = iop.tile([ROWS, FC], f32, tag="mu")
                sg_t = iop.tile([ROWS, FC], f32, tag="sg")
                ep_t = iop.tile([ROWS, FC], f32, tag="ep")
                nc.sync.dma_start(out=mu_t[:], in_=mu_d[:, sl])
                nc.sync.dma_start(out=sg_t[:], in_=sg_d[:, sl])
                nc.sync.dma_start(out=ep_t[:], in_=ep_d[:, sl])

                # prod = (sigma + 1e-5) * eps, rounding exactly as the reference
                prod_t = scr.tile([ROWS, FC], f32, tag="prod")
                nc.vector.scalar_tensor_tensor(
                    out=prod_t[:],
                    in0=sg_t[:],
                    scalar=1e-5,
                    in1=ep_t[:],
                    op0=Alu.add,
                    op1=Alu.mult,
                )
                xi_t = iop.tile([ROWS, FC], f32, tag="xi")
                nc.vector.tensor_add(xi_t[:], prod_t[:], mu_t[:])
                nc.sync.dma_start(out=xi_d[:, sl], in_=xi_t[:])

                # acc_sq[c] = sum(-0.5*eps^2); elementwise out collapsed into a
                # broadcast dummy column (qr.py pattern)
                nc.vector.tensor_tensor_reduce(
                    out=dummy.broadcast_to((ROWS, FC)),
                    in0=ep_t[:],
                    in1=ep_t[:],
                    scale=-0.5,
                    scalar=0.0,
                    op0=Alu.mult,
                    op1=Alu.add,
                    accum_out=sq_part[:, c : c + 1],
                )

                # acc_ln[c] = sum(ln(sigma + 1e-5))
                ln_t = scr.tile([ROWS, FC], f32, tag="ln")
                nc.scalar.activation(
                    out=ln_t[:],
                    in_=sg_t[:],
                    func=mybir.ActivationFunctionType.Ln,
                    bias=bias_t[:],
                    scale=1.0,
                    accum_out=ln_part[:, c : c + 1],
                )

            tot_sq = accp.tile([ROWS, 1], f32, tag="tsq")
            tot_ln = accp.tile([ROWS, 1], f32, tag="tln")
            nc.vector.reduce_sum(tot_sq[:], sq_part[:], axis=mybir.AxisListType.X)
            nc.vector.reduce_sum(tot_ln[:], ln_part[:], axis=mybir.AxisListType.X)
            ll_t = accp.tile([ROWS, 1], f32, tag="ll")
            # ll = (tot_sq - LOG_NORM_TOTAL) - tot_ln
            nc.vector.scalar_tensor_tensor(
                out=ll_t[:],
                in0=tot_sq[:],
                scalar=-LOG_NORM_TOTAL,
                in1=tot_ln[:],
                op0=Alu.add,
                op1=Alu.subtract,
            )
            nc.sync.dma_start(out=ll_d[:], in_=ll_t[:])

    nc.compile()
    return nc


def _get_built():
    global _BUILT
    if _BUILT is None:
        _BUILT = _build()
    return _BUILT


def kernel(mu, sigma, eps):
    from concourse.bass_utils import run_bass_kernel_spmd

    nc = _get_built()

    mu2 = np.ascontiguousarray(np.asarray(mu, dtype=np.float32)).reshape(ROWS_TOTAL, F)
    sg2 = np.ascontiguousarray(np.asarray(sigma, dtype=np.float32)).reshape(ROWS_TOTAL, F)
    ep2 = np.ascontiguousarray(np.asarray(eps, dtype=np.float32)).reshape(ROWS_TOTAL, F)

    in_maps = [
        {
            "mu": mu2[i * ROWS : (i + 1) * ROWS],
            "sigma": sg2[i * ROWS : (i + 1) * ROWS],
            "eps": ep2[i * ROWS : (i + 1) * ROWS],
        }
        for i in range(N_CORES)
    ]

    results = run_bass_kernel_spmd(nc, in_maps, core_ids=list(range(N_CORES))).results

    xi = np.empty((ROWS_TOTAL, F), dtype=np.float32)
    ll = np.empty((ROWS_TOTAL,), dtype=np.float32)
    for i, r in enumerate(results):
        xi[i * ROWS : (i + 1) * ROWS] = r["xi"]
        ll[i * ROWS : (i + 1) * ROWS] = r["ll"].reshape(ROWS)

    return xi.reshape(BS, NCH, TS, D), ll.reshape(BS, NCH)


# revision 7
# speedup vs baseline: 5.8406x; 5.8406x over previous
"""Trainium2 Bass kernel for Calc_Xi_And_LogLikelihood.

reference:
    s  = sigma + 1e-5
    xi = mu + s * eps
    z  = (xi - mu) / s          (== eps up to fp32 rounding noise ~1e-6 rel)
    LL[b,n] = sum_t [ -0.5*sum_d z^2 - sum_d log s - 0.5*D*log(2pi) ]

Sharding: data-parallel over the batch axis. Full shape (64,16,200,64);
each of the 8 cores gets 8 batch rows = a contiguous (128, 12800) f32 slab
(row = one (b,n) pair, free dim = TS*D). Row-wise reductions then map to
per-partition reductions. No cross-core communication.

Per core, per 1600-wide chunk of the free dim:
  DVE : prod = (sigma + 1e-5) * eps           (one scalar_tensor_tensor)
  DVE : xi   = prod + mu                      (tensor_add)      -> DMA out
  DVE : acc_sq[c]  = sum((eps * -0.5) * eps)  (scalar_tensor_tensor w/ accum_out;
                                               tensor_tensor_reduce crashes the
                                               exec unit on this runtime)
  ACT : acc_ln[c]  = sum(ln(sigma + 1e-5))    (activation Ln w/ accum_out)
Finally LL = (sum_c acc_sq - 200*0.5*D*log(2pi)) - sum_c acc_ln.

z = eps is numerically safe: checked against the f64 oracle, the shortcut is
~3.5e-8 max rel err while the fp32 reference itself carries ~1.6e-6.
"""

import math

import numpy as np

N_CORES = 8
BS, NCH, TS, D = 64, 16, 200, 64
ROWS_TOTAL = BS * NCH            # 1024 (b,n) rows overall
ROWS = ROWS_TOTAL // N_CORES     # 128 rows per core -> one SBUF partition each
F = TS * D                       # 12800 contiguous elements per row
FC = 1600                        # free-dim chunk
NCHUNK = F // FC                 # 8
LOG_NORM_TOTAL = TS * 0.5 * D * math.log(2.0 * math.pi)  # 11762.413225019807

_BUILT = None


def _build(reps: int = 1):
    """Build the per-core Bass program. reps>1 repeats the whole compute
    (same data, idempotent) inside one NEFF — used only by test.py to
    measure HW exec time via the wall-clock slope over reps."""
    import concourse.bass as bass
    import concourse.tile as tile
    from concourse import bacc, mybir

    f32 = mybir.dt.float32
    Alu = mybir.AluOpType

    nc = bacc.Bacc(
        "TRN2",
        target_bir_lowering=False,
        debug=False,
        enable_asserts=False,
        num_devices=N_CORES,
    )

    mu_d = nc.dram_tensor("mu", [ROWS, F], f32, kind="ExternalInput").ap()
    sg_d = nc.dram_tensor("sigma", [ROWS, F], f32, kind="ExternalInput").ap()
    ep_d = nc.dram_tensor("eps", [ROWS, F], f32, kind="ExternalInput").ap()
    xi_d = nc.dram_tensor("xi", [ROWS, F], f32, kind="ExternalOutput").ap()
    ll_d = nc.dram_tensor("ll", [ROWS, 1], f32, kind="ExternalOutput").ap()

    with tile.TileContext(nc) as tc:
        with (
            tc.tile_pool(name="io", bufs=3) as iop,
            tc.tile_pool(name="scr", bufs=2) as scr,
            tc.tile_pool(name="acc", bufs=1) as accp,
        ):
            bias_t = accp.tile([ROWS, 1], f32, tag="bias")
            nc.vector.memset(bias_t, 1e-5)
            sq_part = accp.tile([ROWS, NCHUNK], f32, tag="sqp")
            ln_part = accp.tile([ROWS, NCHUNK], f32, tag="lnp")

            for c in range(NCHUNK * reps):
                c = c % NCHUNK
                sl = bass.ts(c, FC)
                mu_t = iop.tile([ROWS, FC], f32, tag="mu")
                sg_t = iop.tile([ROWS, FC], f32, tag="sg")
                ep_t = iop.tile([ROWS, FC], f32, tag="ep")
                nc.sync.dma_start(out=mu_t[:], in_=mu_d[:, sl])
                nc.sync.dma_start(out=sg_t[:], in_=sg_d[:, sl])
                nc.sync.dma_start(out=ep_t[:], in_=ep_d[:, sl])

                # prod = (sigma + 1e-5) * eps, rounding exactly as the reference
                prod_t = scr.tile([ROWS, FC], f32, tag="prod")
                nc.vector.scalar_tensor_tensor(
                    out=prod_t[:],
                    in0=sg_t[:],
                    scalar=1e-5,
                    in1=ep_t[:],
                    op0=Alu.add,
                    op1=Alu.mult,
                )
                xi_t = iop.tile([ROWS, FC], f32, tag="xi")
                nc.vector.tensor_add(xi_t[:], prod_t[:], mu_t[:])
                nc.sync.dma_start(out=xi_d[:, sl], in_=xi_t[:])

                # acc_sq[c] = sum((eps * -0.5) * eps)
                sq_t = scr.tile([ROWS, FC], f32, tag="sq")
                nc.vector.scalar_tensor_tensor(
                    out=sq_t[:],
                    in0=ep_t[:],
                    scalar=-0.5,
                    in1=ep_t[:],
                    op0=Alu.mult,
                    op1=Alu.mult,
                    accum_out=sq_part[:, c : c + 1],
                )

                # acc_ln[c] = sum(ln(sigma + 1e-5))
                ln_t = scr.tile([ROWS, FC], f32, tag="ln")
                nc.scalar.activation(
                    out=ln_t[:],
                    in_=sg_t[:],
                    func=mybir.ActivationFunctionType.Ln,
                    bias=bias_t[:],
                    scale=1.0,
                    accum_out=ln_part[:, c : c + 1],
                )

            tot_sq = accp.tile([ROWS, 1], f32, tag="tsq")
            tot_ln = accp.tile([ROWS, 1], f32, tag="tln")
            nc.vector.reduce_sum(tot_sq[:], sq_part[:], axis=mybir.AxisListType.X)
            nc.vector.reduce_sum(tot_ln[:], ln_part[:], axis=mybir.AxisListType.X)
            ll_t = accp.tile([ROWS, 1], f32, tag="ll")
            # ll = (tot_sq - LOG_NORM_TOTAL) - tot_ln
            nc.vector.scalar_tensor_tensor(
                out=ll_t[:],
                in0=tot_sq[:],
                scalar=-LOG_NORM_TOTAL,
                in1=tot_ln[:],
                op0=Alu.add,
                op1=Alu.subtract,
            )
            nc.sync.dma_start(out=ll_d[:], in_=ll_t[:])

    nc.compile()
    return nc


def _get_built():
    global _BUILT
    if _BUILT is None:
        _BUILT = _build(reps=1)
    return _BUILT


def kernel(mu, sigma, eps):
    from concourse.bass_utils import run_bass_kernel_spmd

    nc = _get_built()

    mu2 = np.ascontiguousarray(np.asarray(mu, dtype=np.float32)).reshape(ROWS_TOTAL, F)
    sg2 = np.ascontiguousarray(np.asarray(sigma, dtype=np.float32)).reshape(ROWS_TOTAL, F)
    ep2 = np.ascontiguousarray(np.asarray(eps, dtype=np.float32)).reshape(ROWS_TOTAL, F)

    in_maps = [
        {
            "mu": mu2[i * ROWS : (i + 1) * ROWS],
            "sigma": sg2[i * ROWS : (i + 1) * ROWS],
            "eps": ep2[i * ROWS : (i + 1) * ROWS],
        }
        for i in range(N_CORES)
    ]

    results = run_bass_kernel_spmd(nc, in_maps, core_ids=list(range(N_CORES))).results

    xi = np.empty((ROWS_TOTAL, F), dtype=np.float32)
    ll = np.empty((ROWS_TOTAL,), dtype=np.float32)
    for i, r in enumerate(results):
        xi[i * ROWS : (i + 1) * ROWS] = r["xi"]
        ll[i * ROWS : (i + 1) * ROWS] = r["ll"].reshape(ROWS)

    return xi.reshape(BS, NCH, TS, D), ll.reshape(BS, NCH)
